# revision 38
# baseline (speedup 1.0000x reference)
"""Trainium2 Bass kernel for nn_Encoder_Decoder_Wrapper (conv encoder -> NTM step -> conv decoder).

Sharding: pure data parallel, batch 64 -> 8 cores x 8 samples. Weights replicated.

Per core, samples are processed in 4 pairs of 2 so every 64-channel conv runs as
K=128/M=128 block-diagonal matmuls (2 samples packed in both contraction and
output partitions).  All conv matmuls use float32r (fp22, 1 cycle/row at N>=256).

Decoder trick: conv3/conv4 read 2x2-upsampled inputs, so each is computed as
4 phase-convs with 2x2 effective kernels over the pre-upsample grid (the phase
weights are sums of adjacent taps, built once from the tap-transposed weights).
This is 16 matmul passes over N/4 columns instead of 9 passes over N columns.

The NTM step is algebraically reduced using its constant initial state:
  - reads0 = h0 = c0 = 0  =>  z = x @ w_lstm_x[:256, (i,g,o)] + b  (f gate unused)
  - memory M == 1e-6 everywhere and the post-read writes are discarded, so
    content addressing of the constant memory gives exactly uniform weights;
    the read vectors collapse to reads[b,h,:] = 1e-6 * S(gamma_h) with
    S = q/(q+1e-8), q = 64*(1/64+1e-16)^gamma, gamma = softplus(clip(p)) + 1,
    where p = h @ w_param[:, 262*h+261] + b_param[262*h+261].
  - their contribution to the output is reads_flat @ w_out[256:], i.e.
    sum_h 1e-6*S_h * colsum_h with colsum_h = w_out[256+256h : 512+256h].sum(0).
"""

import os
import sys

sys.path.insert(0, "/opt/trn_rl_repo")
os.environ.setdefault("MYCRO_LOCAL_CACHE", "1")

import numpy as np

import concourse.bass as bass
import concourse.bacc as bacc
import concourse.mybir as mybir
import concourse.tile as tile
from concourse.masks import make_identity

F32 = mybir.dt.float32
F32R = mybir.dt.float32r
BF16 = mybir.dt.bfloat16
AF = mybir.ActivationFunctionType
ALU = mybir.AluOpType

TAPS = [(dy, dx) for dy in range(3) for dx in range(3)]
LN64 = 4.1588830833596715
CLIP = 20.0

N_CORES = 8
B_CORE = 8          # samples per core
NPAIR = B_CORE // 2


def build_nc(debug=False):
    nc = bacc.Bacc(None, target_bir_lowering=False)

    inp = nc.dram_tensor("inputs", [B_CORE, 1, 64, 64], F32R, kind="ExternalInput")
    wc0 = nc.dram_tensor("w_conv0", [64, 1, 3, 3], F32, kind="ExternalInput")
    bc0 = nc.dram_tensor("b_conv0", [64], F32, kind="ExternalInput")
    wc1 = nc.dram_tensor("w_conv1", [64, 64, 3, 3], F32, kind="ExternalInput")
    bc1 = nc.dram_tensor("b_conv1", [64], F32, kind="ExternalInput")
    wen = nc.dram_tensor("w_enc", [1, 64, 3, 3], F32, kind="ExternalInput")
    ben = nc.dram_tensor("b_enc", [1], F32, kind="ExternalInput")
    wc2 = nc.dram_tensor("w_conv2", [64, 1, 3, 3], F32, kind="ExternalInput")
    bc2 = nc.dram_tensor("b_conv2", [64], F32, kind="ExternalInput")
    wc3 = nc.dram_tensor("w_conv3", [64, 64, 3, 3], F32, kind="ExternalInput")
    bc3 = nc.dram_tensor("b_conv3", [64], F32, kind="ExternalInput")
    wc4 = nc.dram_tensor("w_conv4", [64, 64, 3, 3], F32, kind="ExternalInput")
    bc4 = nc.dram_tensor("b_conv4", [64], F32, kind="ExternalInput")
    wlx = nc.dram_tensor("w_lstm_x", [1024, 1024], F32R, kind="ExternalInput")
    bls = nc.dram_tensor("b_lstm", [1024], F32, kind="ExternalInput")
    wpa = nc.dram_tensor("w_param", [256, 3108], F32R, kind="ExternalInput")
    bpa = nc.dram_tensor("b_param", [3108], F32, kind="ExternalInput")
    wou = nc.dram_tensor("w_out", [1024, 256], F32R, kind="ExternalInput")
    bou = nc.dram_tensor("b_out", [256], F32R, kind="ExternalInput")
    out = nc.dram_tensor("out", [B_CORE, 64, 64, 64], F32, kind="ExternalOutput")

    dbg = {}
    if debug:
        for name, shape, dt in [
            ("dbg_c1in", [128, 34, 34], F32R),
            ("dbg_ein", [128, 18, 18], F32R),
                        ("dbg_h", [128, 2, 8], F32R),
            ("dbg_clip", [B_CORE, 16, 16], F32R),
            ("dbg_lhst2", [4, 8], F32R),
            ("dbg_c3in", [128, 34, 34], F32R),
            ("dbg_x", [B_CORE, 16, 16], F32),
        ]:
            dbg[name] = nc.dram_tensor(name, shape, dt, kind="ExternalOutput")

    with tile.TileContext(nc) as tc:
        with (
            tc.tile_pool(name="const", bufs=1) as const,
            tc.tile_pool(name="work", bufs=1) as work,
            tc.tile_pool(name="dbl", bufs=2) as dbl,
            tc.tile_pool(name="trip", bufs=4) as trip,
            tc.tile_pool(name="quad", bufs=4) as quad,
            tc.tile_pool(name="tri3", bufs=4) as tri3,
            tc.tile_pool(name="psmm", bufs=8, space="PSUM") as psmm,
            tc.tile_pool(name="pssm", bufs=1, space="PSUM") as pssm,
        ):
            # ---------------- setup: identity ----------------
            ident = const.tile([128, 128], F32, tag="ident")
            make_identity(nc, ident)



            # conv2 padded staging rows (one partition per sample); borders
            # zeroed once here, interiors rewritten after the NTM step.
            stg2 = const.tile([8, 21, 19], F32R, tag="stg2")
            nc.vector.memset(stg2[:].bitcast(F32), 0.0)

            # conv0 patches for ALL samples in one tile: partition 16*m+t is
            # sample m shifted by tap t (7 gap partitions per sample zeroed so
            # K=32 matmuls see 0 there), so each tap is a single stride-16
            # partition DMA covering all 8 samples (9 DMAs total).  Pair p
            # sits at base partition 32p, a legal PE tile position.
            pat0 = const.tile([128, 67, 67], F32R, tag="pat0")
            nc.gpsimd.memset(pat0[:, 0:34, :].bitcast(F32), 0.0)
            nc.vector.memset(pat0[:, 34:67, :].bitcast(F32), 0.0)
            # pair-major order so pair 0's nine taps land first and conv0
            # can start while later pairs' patches stream in
            for pp in range(NPAIR):
                for t, (dy, dx) in enumerate(TAPS):
                    eng = (nc.sync, nc.gpsimd, nc.scalar)[t % 3]
                    eng.dma_start(
                        out=bass.AP(
                            tensor=pat0[:].tensor,
                            offset=pat0[:].offset
                            + (32 * pp + t) * 4489 + (3 - dy) * 67 + (3 - dx),
                            ap=[[16 * 4489, 2], [67, 64], [1, 64]],
                        ),
                        in_=bass.AP(
                            tensor=inp[:].tensor,
                            offset=2 * pp * 4096,
                            ap=[[4096, 2], [64, 64], [1, 64]],
                        ),
                    )


            # ---------------- 1ch conv weights -> [128,128] lhsT -----------
            # rows 16*s+t (t<9) hold w.T for sample half s; zero elsewhere;
            # replicated at partition bases 32/64/96 so each pair's K=32
            # matmul has lhsT at its own base.
            convT = {}
            for name, wdram in (("c0", wc0), ("c2", wc2)):
                s9 = const.tile([64, 9], F32, tag=f"w9_{name}")
                nc.sync.dma_start(
                    out=s9[:], in_=wdram[:].rearrange("a b c d -> a (b c d)")
                )
                ct = const.tile([128, 128], F32R, tag=f"cT_{name}")
                nc.vector.memset(ct[0:32, :].bitcast(F32), 0.0)
                p9 = psmm.tile([9, 64], F32, tag="mm")
                nc.tensor.transpose(p9[:], s9[:], ident[0:64, 0:64])
                nc.scalar.activation(ct[0:9, 0:64], p9[:], AF.Copy, bias=0.0, scale=1.0)
                nc.gpsimd.dma_start(out=ct[16:25, 64:128], in_=ct[0:9, 0:64])
                for b, eng in ((32, nc.gpsimd), (64, nc.sync), (96, nc.scalar)):
                    eng.dma_start(out=ct[b : b + 32, :], in_=ct[0:32, :])
                convT[name] = ct

            # ---------------- 64ch conv weights -> block-diag lhsT ---------
            # wtap[name][:, t, :] is the [128,128] lhsT for tap t:
            #   rows 0:64  = w.T[ci,co] in cols 0:64   (sample A)
            #   rows 64:128= w.T[ci,co] in cols 64:128 (sample B)
            wtap = {}

            def build_wtap(name, wdram, scale):
                wsrc = dbl.tile([64, 576], F32, tag="c4in")  # aliased slot
                nc.sync.dma_start(
                    out=wsrc[:], in_=wdram[:].rearrange("a b c d -> a (b c d)")
                )
                wt = const.tile([128, 9, 128], F32R, tag=f"wtap_{name}")
                nc.vector.memset(wt[:].bitcast(F32), 0.0)
                for t in range(9):
                    pw = psmm.tile([64, 64], F32, tag="mm")
                    nc.tensor.transpose(pw[:], wsrc[:, t::9], ident[0:64, 0:64])
                    nc.scalar.activation(
                        wt[0:64, t, 0:64], pw[:], AF.Copy, bias=0.0, scale=scale
                    )
                nc.gpsimd.dma_start(out=wt[64:128, :, 64:128], in_=wt[0:64, :, 0:64])
                wtap[name] = wt

            build_wtap("c1", wc1, 0.25)  # 0.25: preceding avg-pool folded in

            # enc conv (64ci -> 1co): lhsT[:, t, :] is [128, 2]
            wencs = const.tile([64, 9], F32, tag="wencs")
            nc.sync.dma_start(
                out=wencs[:], in_=wen[:].rearrange("a b c d -> (a b) (c d)")
            )
            encT = const.tile([128, 9, 2], F32R, tag="encT")
            nc.vector.memset(encT[:].bitcast(F32), 0.0)
            for t in range(9):
                nc.scalar.activation(
                    encT[0:64, t, 0:1],
                    wencs[:, t : t + 1],
                    AF.Copy,
                    bias=0.0,
                    scale=0.25,  # preceding avg-pool folded in
                )
            nc.gpsimd.dma_start(out=encT[64:128, :, 1:2], in_=encT[0:64, :, 0:1])

            # ---------------- conv biases -> [128,1] (both sample halves) ---
            def bias128(dram_b, tag):
                bt = const.tile([128, 1], F32, tag=tag)
                nc.sync.dma_start(out=bt[0:64, :], in_=dram_b[:].unsqueeze(1))
                nc.sync.dma_start(out=bt[64:128, :], in_=dram_b[:].unsqueeze(1))
                return bt

            bt0 = bias128(bc0, "bt0")
            bt1 = bias128(bc1, "bt1")
            bt2 = bias128(bc2, "bt2")
            bt3 = bias128(bc3, "bt3")
            bt4 = bias128(bc4, "bt4")
            bte = const.tile([2, 1], F32, tag="bte")
            nc.sync.dma_start(
                out=bte[:],
                in_=bass.AP(tensor=ben[:].tensor, offset=0, ap=[[0, 2], [1, 1]]),
            )

            xstage = const.tile([8, 16, 16], F32, tag="xstage")

            # ================ encoder: stage-major over 4 sample pairs ======
            # conv0 phase: dense matmuls for all pairs; relu-evict (ACT) and
            # 2x2 pool (two DVE adds) chase per tile, writing straight into
            # the padded conv1 input.
            c1in_l = []
            for p in range(NPAIR):
                c1in = tri3.tile([128, 34, 34], F32R, tag="c1in")
                nc.gpsimd.memset(c1in[:, 0:1, :].bitcast(F32), 0.0)
                nc.gpsimd.memset(c1in[:, 33:34, :].bitcast(F32), 0.0)
                nc.gpsimd.memset(c1in[:, 1:33, 0:1].bitcast(F32), 0.0)
                nc.gpsimd.memset(c1in[:, 1:33, 33:34].bitcast(F32), 0.0)
                for n in range(8):
                    ps = psmm.tile([128, 4, 2, 32, 2], F32, tag="mm")
                    nc.tensor.matmul(
                        ps[:],
                        convT["c0"][32 * p : 32 * p + 32, :],
                        pat0[32 * p : 32 * p + 32, 2 + n * 8 : 10 + n * 8, 2:66],
                        start=True,
                        stop=True,
                        tile_position=(32 * p, 0),
                    )
                    ct0 = trip.tile([128, 4, 2, 32, 2], F32, tag="ct0")
                    nc.scalar.activation(ct0[:], ps[:], AF.Relu, bias=bt0)
                    tcol = tri3.tile([128, 4, 2, 32], F32, tag="tcol")
                    nc.vector.tensor_add(
                        tcol[:], ct0[:, :, :, :, 0], ct0[:, :, :, :, 1]
                    )
                    nc.vector.tensor_add(
                        c1in[:, 1 + 4 * n : 5 + 4 * n, 1:33],
                        tcol[:, :, 0, :],
                        tcol[:, :, 1, :],
                    )
                c1in_l.append(c1in)
            # conv1 phase
            ein_l = []
            for p in range(NPAIR):
                c1in = c1in_l[p]
                e_in = quad.tile([128, 18, 18], F32R, tag="e_in")
                nc.gpsimd.memset(e_in[:, 0:1, :].bitcast(F32), 0.0)
                nc.gpsimd.memset(e_in[:, 17:18, :].bitcast(F32), 0.0)
                nc.gpsimd.memset(e_in[:, 1:17, 0:1].bitcast(F32), 0.0)
                nc.gpsimd.memset(e_in[:, 1:17, 17:18].bitcast(F32), 0.0)
                for n in range(2):
                    ps = psmm.tile([128, 8, 2, 16, 2], F32, tag="mm")
                    for t, (dy, dx) in enumerate(TAPS):
                        nc.tensor.matmul(
                            ps[:],
                            wtap["c1"][:, t, :],
                            c1in[:, n * 16 + dy : n * 16 + dy + 16, dx : dx + 32]
                            ,
                            start=(t == 0),
                            stop=(t == 8),
                        )
                    ct1 = trip.tile([128, 8, 2, 16, 2], F32, tag="ct1")
                    nc.scalar.activation(ct1[:], ps[:], AF.Relu, bias=bt1)
                    tc1 = tri3.tile([128, 8, 2, 16], F32, tag="tc1")
                    nc.vector.tensor_add(
                        tc1[:], ct1[:, :, :, :, 0], ct1[:, :, :, :, 1]
                    )
                    nc.vector.tensor_add(
                        e_in[:, 1 + 8 * n : 9 + 8 * n, 1:17],
                        tc1[:, :, 0, :],
                        tc1[:, :, 1, :],
                    )
                ein_l.append(e_in)
            # enc phase
            for p in range(NPAIR):
                e_in = ein_l[p]
                pe = psmm.tile([2, 16, 16], F32, tag="mm")
                for t, (dy, dx) in enumerate(TAPS):
                    nc.tensor.matmul(
                        pe[:],
                        encT[:, t, :],
                        e_in[:, dy : dy + 16, dx : dx + 16],
                        start=(t == 0),
                        stop=(t == 8),
                    )
                estage = dbl.tile([2, 16, 16], F32, tag="estage")
                nc.scalar.activation(estage[:], pe[:], AF.Relu, bias=bte)
                nc.scalar.dma_start(out=xstage[2 * p : 2 * p + 2, :, :], in_=estage[:])

            # pre-warm the ACT tables the NTM step needs so the loads happen
            # during the encoder tail instead of inside the NTM bubble
            warm = const.tile([1, 4], F32, tag="warm")
            for wi, af in enumerate((AF.Sigmoid, AF.Tanh, AF.Exp, AF.Ln)):
                nc.scalar.activation(
                    warm[:, wi : wi + 1], ident[0:1, 0:1], af, bias=0.0
                )

            # deferred weight prep: decoder taps + NTM weights (fills the
            # PE bubble while the NTM chain runs)
            build_wtap("c3", wc3, 1.0)
            build_wtap("c4", wc4, 1.0)

            # ---------------- phase-conv weights for c3/c4 ------------------
            # conv3/conv4 inputs are 2x2 upsamples, so conv = 4 phase convs
            # with 2x2 kernels whose taps are sums of adjacent 3x3 taps:
            #   y-variants: V1 = dy1+dy2, V2 = dy0+dy1 (V0=dy0, V3=dy2 are
            #   original taps); same for x.  Phase (r=0) uses (V0,off 0),
            #   (V1,off 1); phase (r=1) uses (V2,off 1), (V3,off 2).
            PH = {0: [(0, 0), (1, 1)], 1: [(2, 1), (3, 2)]}
            phw = {}
            for name in ("c3", "c4"):
                wt3 = wtap[name][:].rearrange("p (a b) m -> p a b m", a=3)
                tA = const.tile([128, 2, 3, 128], F32R, tag=f"tA_{name}")
                tB = const.tile([128, 3, 2, 128], F32R, tag=f"tB_{name}")
                tC = const.tile([128, 2, 2, 128], F32R, tag=f"tC_{name}")
                nc.vector.tensor_add(tA[:, 0], wt3[:, 1], wt3[:, 2])
                nc.vector.tensor_add(tA[:, 1], wt3[:, 0], wt3[:, 1])
                nc.gpsimd.tensor_add(tB[:, :, 0], wt3[:, :, 1], wt3[:, :, 2])
                nc.gpsimd.tensor_add(tB[:, :, 1], wt3[:, :, 0], wt3[:, :, 1])
                nc.vector.tensor_add(tC[:, :, 0], tA[:, :, 1], tA[:, :, 2])
                nc.vector.tensor_add(tC[:, :, 1], tA[:, :, 0], tA[:, :, 1])
                phw[name] = (wt3, tA, tB, tC)

            def phsel(name, v, u):
                wt3, tA, tB, tC = phw[name]
                vy = {0: 0, 3: 2}.get(v)
                ux = {0: 0, 3: 2}.get(u)
                if vy is not None and ux is not None:
                    return wt3[:, vy, ux, :]
                if vy is not None:
                    return tB[:, vy, u - 1, :]
                if ux is not None:
                    return tA[:, v - 1, ux, :]
                return tC[:, v - 1, u - 1, :]
            # ---------------- NTM weights ----------------------------------
            # w_lstm_x rows 0:256 for gates (i, g, o); k-tiled in partitions.
            # batched: one DMA for gate i (cols 0:256), one for g+o (512:1024)
            wx = const.tile([128, 2, 768], F32R, tag="wx")
            nc.scalar.dma_start(
                out=bass.AP(
                    tensor=wx[:].tensor, offset=wx[:].offset,
                    ap=[[1536, 128], [768, 2], [1, 256]],
                ),
                in_=bass.AP(
                    tensor=wlx[:].tensor, offset=0,
                    ap=[[1024, 128], [131072, 2], [1, 256]],
                ),
            )
            nc.scalar.dma_start(
                out=bass.AP(
                    tensor=wx[:].tensor, offset=wx[:].offset + 256,
                    ap=[[1536, 128], [768, 2], [1, 512]],
                ),
                in_=bass.AP(
                    tensor=wlx[:].tensor, offset=512,
                    ap=[[1024, 128], [131072, 2], [1, 512]],
                ),
            )
            bigo = const.tile([128, 6], F32, tag="bigo")
            for j, c0 in enumerate([0, 512, 768]):
                for h2 in range(2):
                    nc.scalar.dma_start(
                        out=bigo[:, j * 2 + h2 : j * 2 + h2 + 1],
                        in_=bls[c0 + h2 * 128 : c0 + (h2 + 1) * 128].unsqueeze(1),
                    )
            # w_param gamma columns {262h+261}
            wp3 = const.tile([128, 2, 3], F32R, tag="wp3")
            for kt in range(2):
                nc.gpsimd.dma_start(
                    out=wp3[:, kt, :],
                    in_=bass.AP(
                        tensor=wpa[:].tensor,
                        offset=kt * 128 * 3108 + 261,
                        ap=[[3108, 128], [262, 3]],
                    ),
                )
            bp3 = const.tile([3, 1], F32, tag="bp3")
            nc.sync.dma_start(
                out=bp3[:],
                in_=bass.AP(tensor=bpa[:].tensor, offset=261, ap=[[262, 3], [1, 1]]),
            )
            # w_out rows 0:256 (h part) and 256:1024 (reads part, for colsums)
            wo = const.tile([128, 2, 256], F32R, tag="wo")
            nc.scalar.dma_start(
                out=bass.AP(
                    tensor=wo[:].tensor, offset=wo[:].offset,
                    ap=[[512, 128], [256, 2], [1, 256]],
                ),
                in_=bass.AP(
                    tensor=wou[:].tensor, offset=0,
                    ap=[[256, 128], [32768, 2], [1, 256]],
                ),
            )
            w2c = dbl.tile([128, 6, 256], F32R, tag="w2c")
            nc.sync.dma_start(
                out=bass.AP(
                    tensor=w2c[:].tensor, offset=w2c[:].offset,
                    ap=[[1536, 128], [256, 6], [1, 256]],
                ),
                in_=bass.AP(
                    tensor=wou[:].tensor, offset=65536,
                    ap=[[256, 128], [32768, 6], [1, 256]],
                ),
            )
            ones3 = const.tile([128, 6, 3], F32R, tag="ones3")
            nc.vector.memset(ones3[:].bitcast(F32), 0.0)
            for c in range(6):
                nc.vector.memset(ones3[:, c, c // 2 : c // 2 + 1].bitcast(F32), 1.0)
            # rhs2: rows 0:3 = per-head colsums of w_out reads part, row 3 = b_out
            rhs2 = const.tile([4, 256], F32R, tag="rhs2")
            nc.scalar.dma_start(out=rhs2[3:4, :], in_=bou[:].unsqueeze(0))
            pcs = psmm.tile([3, 256], F32, tag="mm")
            for c in range(6):
                nc.tensor.matmul(
                    pcs[:],
                    ones3[:, c, :],
                    w2c[:, c, :],
                    start=(c == 0),
                    stop=(c == 5),
                )
            nc.scalar.activation(rhs2[0:3, :], pcs[:], AF.Copy, bias=0.0, scale=1.0)
            # lhsT2: rows 0:3 = 1e-6 * S(gamma) (filled later), row 3 = 1 (bias)
            lhsT2 = const.tile([4, 8], F32R, tag="lhsT2")
            nc.vector.memset(lhsT2[:].bitcast(F32), 1.0)  # rows 0:3 rewritten before use

            # ================ NTM step (all 8 samples at once) ==============
            if debug:
                nc.sync.dma_start(out=dbg["dbg_x"][:], in_=xstage[:])
            # x^T k-tiles via PE transpose
            xT = work.tile([128, 2, 8], F32R, tag="xT")
            for kt in range(2):
                pxt = psmm.tile([128, 8], F32, tag="mm")
                nc.tensor.transpose(
                    pxt[:],
                    xstage[:].rearrange("p a b -> p (a b)")[:, kt * 128 : kt * 128 + 128],
                    ident[0:8, 0:8],
                )
                nc.scalar.activation(xT[:, kt, :], pxt[:], AF.Copy, bias=0.0, scale=1.0)
            # z = x @ Wx + b for gates i, g, o; h = sig(o) * tanh(sig(i)*tanh(g))
            zps = psmm.tile([128, 6, 8], F32, tag="mm")
            for j in range(3):
                for h2 in range(2):
                    for kt in range(2):
                        nc.tensor.matmul(
                            zps[:, 2 * j + h2, :],
                            wx[:, kt, j * 256 + h2 * 128 : j * 256 + h2 * 128 + 128],
                            xT[:, kt, :],
                            start=(kt == 0),
                            stop=(kt == 1),
                        )
            zb = work.tile([128, 6, 8], F32, tag="zb")
            bigo_b = bass.AP(
                tensor=bigo[:].tensor, offset=bigo[:].offset,
                ap=[list(d) for d in bigo[:].ap] + [[0, 8]],
            )
            nc.vector.tensor_tensor(zb[:], zps[:], bigo_b, op=ALU.add)
            si = work.tile([128, 2, 8], F32, tag="gate0")
            nc.scalar.activation(si[:], zb[:, 0:2, :], AF.Sigmoid, bias=0.0)
            tg = work.tile([128, 2, 8], F32, tag="gate1")
            nc.scalar.activation(tg[:], zb[:, 2:4, :], AF.Tanh, bias=0.0)
            so = work.tile([128, 2, 8], F32, tag="gate2")
            nc.scalar.activation(so[:], zb[:, 4:6, :], AF.Sigmoid, bias=0.0)
            ctile = work.tile([128, 2, 8], F32, tag="ctile")
            nc.vector.tensor_mul(ctile[:], si[:], tg[:])
            tct = work.tile([128, 2, 8], F32, tag="tct")
            nc.scalar.activation(tct[:], ctile[:], AF.Tanh, bias=0.0)
            h = work.tile([128, 2, 8], F32R, tag="h")
            nc.vector.tensor_mul(h[:], so[:], tct[:])
            if debug:
                nc.sync.dma_start(out=dbg["dbg_h"][:], in_=h[:])
            # gamma path: p3 = clip(h @ wp3 + bp3); q = 64*(1/64+1e-16)^gamma
            pp3 = psmm.tile([3, 8], F32, tag="mm")
            for kt in range(2):
                nc.tensor.matmul(
                    pp3[:], wp3[:, kt, :], h[:, kt, :], start=(kt == 0), stop=(kt == 1)
                )
            t1 = work.tile([3, 8], F32, tag="t1")
            nc.scalar.activation(t1[:], pp3[:], AF.Identity, bias=bp3)
            t2 = work.tile([3, 8], F32, tag="t2")
            nc.vector.tensor_scalar(t2[:], t1[:], -CLIP, CLIP, ALU.max, ALU.min)
            # softplus(p) = ln(1+exp(p)); gamma = softplus + 1,
            # q = 64*(1/64)^gamma = exp(-softplus(p)*ln64)
            eu = work.tile([3, 8], F32, tag="eu")
            nc.scalar.activation(eu[:], t2[:], AF.Exp, bias=0.0)
            ev = work.tile([3, 8], F32, tag="ev")
            nc.vector.tensor_scalar_add(ev[:], eu[:], 1.0)
            sp = work.tile([3, 8], F32, tag="sp")
            nc.scalar.activation(sp[:], ev[:], AF.Ln, bias=0.0)
            q = work.tile([3, 8], F32, tag="q")
            nc.scalar.activation(q[:], sp[:], AF.Exp, bias=0.0, scale=-LN64)
            qe = work.tile([3, 8], F32, tag="qe")
            nc.vector.tensor_scalar_add(qe[:], q[:], 1e-8)
            rec = work.tile([3, 8], F32, tag="rec")
            nc.vector.reciprocal(rec[:], qe[:])
            # lhsT2 rows 0:3 = 1e-6 * q / (q + 1e-8)
            nc.vector.scalar_tensor_tensor(
                out=lhsT2[0:3, :], in0=q[:], scalar=1e-6, in1=rec[:],
                op0=ALU.mult, op1=ALU.mult,
            )
            if debug:
                nc.sync.dma_start(out=dbg["dbg_lhst2"][:], in_=lhsT2[:])
            # out = clip(h @ w_out[:256] + reads @ w_out[256:] + b_out)
            pout = psmm.tile([8, 16, 16], F32, tag="mm")
            for kt in range(2):
                nc.tensor.matmul(
                    pout[:].rearrange("p a b -> p (a b)"),
                    h[:, kt, :],
                    wo[:, kt, :],
                    start=(kt == 0),
                    stop=False,
                )
            nc.tensor.matmul(
                pout[:].rearrange("p a b -> p (a b)"),
                lhsT2[:],
                rhs2[:],
                start=False,
                stop=True,
            )
            nc.vector.tensor_scalar(
                stg2[:, 1:17, 1:17], pout[:], -CLIP, CLIP, ALU.max, ALU.min
            )
            if debug:
                nc.sync.dma_start(out=dbg["dbg_clip"][:], in_=stg2[:, 1:17, 1:17])

            # ================ decoder: 4 sample pairs =======================
            # conv2 patches for all samples in one tile (partition 16*m+tap):
            # one stride-16 partition DMA per tap (9 DMAs total); gap
            # partitions were zeroed at setup.
            pc2 = const.tile([128, 18, 19], F32R, tag="pc2")
            nc.gpsimd.memset(pc2[:].bitcast(F32), 0.0)
            for dy in range(3):
                for k in range(3):
                    eng = (nc.sync, nc.gpsimd, nc.scalar)[(3 * dy + k) % 3]
                    eng.dma_start(
                        out=bass.AP(
                            tensor=pc2[:].tensor,
                            offset=pc2[:].offset + (3 * dy + k) * 342,
                            ap=[[16 * 342, 8], [1, 341]],
                        ),
                        in_=bass.AP(
                            tensor=stg2[:].tensor,
                            offset=stg2[:].offset + dy * 19 + k,
                            ap=[[399, 8], [1, 341]],
                        ),
                    )
            for p in range(NPAIR):
                # --- conv2: K=32 single matmul, N=256 -> v2 [128,18,18] pad
                ps2 = psmm.tile([128, 16, 16], F32, tag="mm")
                nc.tensor.matmul(
                    ps2[:],
                    convT["c2"][32 * p : 32 * p + 32, :],
                    pc2[32 * p : 32 * p + 32, 0:16, 0:16],
                    start=True,
                    stop=True,
                    tile_position=(32 * p, 0),
                )
                v2 = quad.tile([128, 18, 18], F32R, tag="v2")
                nc.gpsimd.memset(v2[:, 0:1, :].bitcast(F32), 0.0)
                nc.gpsimd.memset(v2[:, 17:18, :].bitcast(F32), 0.0)
                nc.gpsimd.memset(v2[:, 1:17, 0:1].bitcast(F32), 0.0)
                nc.gpsimd.memset(v2[:, 1:17, 17:18].bitcast(F32), 0.0)
                nc.scalar.activation(v2[:, 1:17, 1:17], ps2[:], AF.Relu, bias=bt2)
                # --- conv3 as 4 phase convs (input is 2x2 upsample of v2)
                v3 = dbl.tile([128, 17, 2, 17, 2], F32R, tag="v3")
                nc.vector.memset(v3[:, 0, 0, :, :].bitcast(F32), 0.0)       # row 0
                nc.vector.memset(v3[:, 16, 1, :, :].bitcast(F32), 0.0)      # row 33
                nc.vector.memset(v3[:, :, :, 0, 0].bitcast(F32), 0.0)       # col 0
                nc.vector.memset(v3[:, :, :, 16, 1].bitcast(F32), 0.0)      # col 33
                for ph, (ry, rx) in enumerate([(0, 0), (0, 1), (1, 0), (1, 1)]):
                    ps = psmm.tile([128, 16, 16], F32, tag="mm")
                    i = 0
                    for v, yo in PH[ry]:
                        for u, xo in PH[rx]:
                            nc.tensor.matmul(
                                ps[:],
                                phsel("c3", v, u),
                                v2[:, yo : yo + 16, xo : xo + 16],
                                start=(i == 0),
                                stop=(i == 3),
                            )
                            i += 1
                    dst = v3[:, ry : 16 + ry, 1 - ry, rx : 16 + rx, 1 - rx]
                    if ph % 2 == 0:
                        nc.scalar.activation(dst, ps[:], AF.Relu, bias=bt3)
                    else:
                        nc.vector.tensor_scalar(
                            dst, ps[:], bt3[:], 0.0, ALU.add, ALU.max
                        )
                v3f = v3[:].rearrange("p r a c b -> p (r a) (c b)")
                if debug and p == 0:
                    nc.sync.dma_start(out=dbg["dbg_c3in"][:], in_=v3f)
                # --- conv4 as 4 phase convs (input is 2x2 upsample of v3)
                c4out = dbl.tile([128, 32, 2, 32, 2], F32, tag="c4out")
                ei = 0
                for sy in (0, 1):
                    for h in (0, 1):
                        for sx in (0, 1):
                            ps = psmm.tile([128, 16, 32], F32, tag="mm")
                            i = 0
                            for v, yo in PH[sy]:
                                for u, xo in PH[sx]:
                                    nc.tensor.matmul(
                                        ps[:],
                                        phsel("c4", v, u),
                                        v3f[:, yo + 16 * h : yo + 16 * h + 16, xo : xo + 32],
                                        start=(i == 0),
                                        stop=(i == 3),
                                    )
                                    i += 1
                            dst = c4out[:, 16 * h : 16 * h + 16, sy, :, sx]
                            if ei % 2 == 0:
                                nc.scalar.activation(dst, ps[:], AF.Relu, bias=bt4)
                            else:
                                nc.vector.tensor_scalar(
                                    dst, ps[:], bt4[:], 0.0, ALU.add, ALU.max
                                )
                            ei += 1
                        if p == NPAIR - 1:
                            # fine-grained row stores so the tail drains early:
                            # rows 32h+sy(+2k) for k<16 are complete now
                            for s01 in range(2):
                                for c in range(4):
                                    eng = (nc.sync, nc.gpsimd, nc.scalar)[(s01 * 4 + c) % 3]
                                    eng.dma_start(
                                        out=bass.AP(
                                            tensor=out[:].tensor,
                                            offset=(2 * p + s01) * 262144
                                            + (16 * c) * 4096
                                            + (32 * h + sy) * 64,
                                            ap=[[4096, 16], [128, 16], [1, 64]],
                                        ),
                                        in_=c4out[
                                            64 * s01 + 16 * c : 64 * s01 + 16 * c + 16,
                                            16 * h : 16 * h + 16, sy, :, :,
                                        ],
                                    )
                if p < NPAIR - 1:
                    c4v = c4out[:].rearrange("p r a c b -> p (r a) (c b)")
                    for s01 in range(2):
                        for c in range(4):
                            eng = (nc.sync, nc.gpsimd)[(s01 * 4 + c) % 2]
                            eng.dma_start(
                                out=out[2 * p + s01, 16 * c : 16 * c + 16, :, :],
                                in_=c4v[64 * s01 + 16 * c : 64 * s01 + 16 * c + 16, :, :],
                            )


    nc.compile()
    return nc


_NC_CACHE = {}
LAST_RESULT = None

WEIGHT_NAMES = [
    "w_conv0", "b_conv0", "w_conv1", "b_conv1", "w_enc", "b_enc",
    "w_conv2", "b_conv2", "w_conv3", "b_conv3", "w_conv4", "b_conv4",
    "w_lstm_x", "b_lstm", "w_param", "b_param", "w_out", "b_out",
]


def kernel(**inputs):
    global LAST_RESULT
    from concourse.bass_utils import run_bass_kernel_spmd

    debug = bool(int(os.environ.get("KDEBUG", "0")))
    key = ("nc", debug)
    if key not in _NC_CACHE:
        _NC_CACHE[key] = build_nc(debug=debug)
    nc = _NC_CACHE[key]

    xs = np.ascontiguousarray(np.asarray(inputs["inputs"], dtype=np.float32))
    weights = {
        k: np.ascontiguousarray(np.asarray(inputs[k], dtype=np.float32))
        for k in WEIGHT_NAMES
    }
    in_maps = []
    for c in range(N_CORES):
        m = dict(weights)
        m["inputs"] = xs[c * B_CORE : (c + 1) * B_CORE]
        in_maps.append(m)

    res = run_bass_kernel_spmd(nc, in_maps, core_ids=list(range(N_CORES)))
    LAST_RESULT = res
    return np.concatenate([r["out"] for r in res.results], axis=0)


if __name__ == "__main__":
    nc = build_nc()
    print("built ok")



# revision 42
# speedup vs baseline: 1.0056x; 1.0056x over previous
"""Trainium2 Bass kernel for nn_Encoder_Decoder_Wrapper (conv encoder -> NTM step -> conv decoder).

Sharding: pure data parallel, batch 64 -> 8 cores x 8 samples. Weights replicated.

Per core, samples are processed in 4 pairs of 2 so every 64-channel conv runs as
K=128/M=128 block-diagonal matmuls (2 samples packed in both contraction and
output partitions).  All conv matmuls use float32r (fp22, 1 cycle/row at N>=256).

Decoder trick: conv3/conv4 read 2x2-upsampled inputs, so each is computed as
4 phase-convs with 2x2 effective kernels over the pre-upsample grid (the phase
weights are sums of adjacent taps, built once from the tap-transposed weights).
This is 16 matmul passes over N/4 columns instead of 9 passes over N columns.

The NTM step is algebraically reduced using its constant initial state:
  - reads0 = h0 = c0 = 0  =>  z = x @ w_lstm_x[:256, (i,g,o)] + b  (f gate unused)
  - memory M == 1e-6 everywhere and the post-read writes are discarded, so
    content addressing of the constant memory gives exactly uniform weights;
    the read vectors collapse to reads[b,h,:] = 1e-6 * S(gamma_h) with
    S = q/(q+1e-8), q = 64*(1/64+1e-16)^gamma, gamma = softplus(clip(p)) + 1,
    where p = h @ w_param[:, 262*h+261] + b_param[262*h+261].
  - their contribution to the output is reads_flat @ w_out[256:], i.e.
    sum_h 1e-6*S_h * colsum_h with colsum_h = w_out[256+256h : 512+256h].sum(0).
"""

import os
import sys

sys.path.insert(0, "/opt/trn_rl_repo")
os.environ.setdefault("MYCRO_LOCAL_CACHE", "1")

import numpy as np

import concourse.bass as bass
import concourse.bacc as bacc
import concourse.mybir as mybir
import concourse.tile as tile
from concourse.masks import make_identity

F32 = mybir.dt.float32
F32R = mybir.dt.float32r
BF16 = mybir.dt.bfloat16
AF = mybir.ActivationFunctionType
ALU = mybir.AluOpType

TAPS = [(dy, dx) for dy in range(3) for dx in range(3)]
LN64 = 4.1588830833596715
CLIP = 20.0

N_CORES = 8
B_CORE = 8          # samples per core
NPAIR = B_CORE // 2


def build_nc(debug=False):
    nc = bacc.Bacc(None, target_bir_lowering=False)

    inp = nc.dram_tensor("inputs", [B_CORE, 1, 64, 64], F32R, kind="ExternalInput")
    wc0 = nc.dram_tensor("w_conv0", [64, 1, 3, 3], F32, kind="ExternalInput")
    bc0 = nc.dram_tensor("b_conv0", [64], F32, kind="ExternalInput")
    wc1 = nc.dram_tensor("w_conv1", [64, 64, 3, 3], F32, kind="ExternalInput")
    bc1 = nc.dram_tensor("b_conv1", [64], F32, kind="ExternalInput")
    wen = nc.dram_tensor("w_enc", [1, 64, 3, 3], F32, kind="ExternalInput")
    ben = nc.dram_tensor("b_enc", [1], F32, kind="ExternalInput")
    wc2 = nc.dram_tensor("w_conv2", [64, 1, 3, 3], F32, kind="ExternalInput")
    bc2 = nc.dram_tensor("b_conv2", [64], F32, kind="ExternalInput")
    wc3 = nc.dram_tensor("w_conv3", [64, 64, 3, 3], F32, kind="ExternalInput")
    bc3 = nc.dram_tensor("b_conv3", [64], F32, kind="ExternalInput")
    wc4 = nc.dram_tensor("w_conv4", [64, 64, 3, 3], F32, kind="ExternalInput")
    bc4 = nc.dram_tensor("b_conv4", [64], F32, kind="ExternalInput")
    wlx = nc.dram_tensor("w_lstm_x", [1024, 1024], F32R, kind="ExternalInput")
    bls = nc.dram_tensor("b_lstm", [1024], F32, kind="ExternalInput")
    wpa = nc.dram_tensor("w_param", [256, 3108], F32R, kind="ExternalInput")
    bpa = nc.dram_tensor("b_param", [3108], F32, kind="ExternalInput")
    wou = nc.dram_tensor("w_out", [1024, 256], F32R, kind="ExternalInput")
    bou = nc.dram_tensor("b_out", [256], F32R, kind="ExternalInput")
    out = nc.dram_tensor("out", [B_CORE, 64, 64, 64], F32, kind="ExternalOutput")

    dbg = {}
    if debug:
        for name, shape, dt in [
            ("dbg_c1in", [128, 34, 34], F32R),
            ("dbg_ein", [128, 18, 18], F32R),
                        ("dbg_h", [128, 2, 8], F32R),
            ("dbg_clip", [B_CORE, 16, 16], F32R),
            ("dbg_lhst2", [4, 8], F32R),
            ("dbg_c3in", [128, 34, 34], F32R),
            ("dbg_x", [B_CORE, 16, 16], F32),
        ]:
            dbg[name] = nc.dram_tensor(name, shape, dt, kind="ExternalOutput")

    with tile.TileContext(nc) as tc:
        with (
            tc.tile_pool(name="const", bufs=1) as const,
            tc.tile_pool(name="work", bufs=1) as work,
            tc.tile_pool(name="dbl", bufs=2) as dbl,
            tc.tile_pool(name="trip", bufs=4) as trip,
            tc.tile_pool(name="quad", bufs=4) as quad,
            tc.tile_pool(name="tri3", bufs=4) as tri3,
            tc.tile_pool(name="psmm", bufs=8, space="PSUM") as psmm,
            tc.tile_pool(name="pssm", bufs=1, space="PSUM") as pssm,
        ):
            # ---------------- setup: identity ----------------
            ident = const.tile([128, 128], F32, tag="ident")
            make_identity(nc, ident)



            # conv2 padded staging rows (one partition per sample); borders
            # zeroed once here, interiors rewritten after the NTM step.
            stg2 = const.tile([8, 21, 19], F32R, tag="stg2")
            nc.vector.memset(stg2[:].bitcast(F32), 0.0)

            # conv0 patches for ALL samples in one tile: pair p occupies
            # partitions 32p..32p+17 (partition 32p+9s+t = sample 2p+s shifted
            # by tap t), so K=18 matmuls at base 32p never see the unused
            # partitions (no zero fill needed) and each (pair, tap) is one
            # stride-9 two-sample DMA.  Pair-major so pair 0 lands first.
            pat0 = const.tile([128, 67, 67], F32R, tag="pat0")
            nc.vector.memset(pat0[:, 0:3, :].bitcast(F32), 0.0)
            nc.vector.memset(pat0[:, 65:67, :].bitcast(F32), 0.0)
            nc.vector.memset(pat0[:, 3:65, 0:3].bitcast(F32), 0.0)
            nc.vector.memset(pat0[:, 3:65, 65:67].bitcast(F32), 0.0)
            for pp in range(NPAIR):
                for t, (dy, dx) in enumerate(TAPS):
                    eng = (nc.sync, nc.gpsimd, nc.scalar)[t % 3]
                    eng.dma_start(
                        out=bass.AP(
                            tensor=pat0[:].tensor,
                            offset=pat0[:].offset
                            + (32 * pp + t) * 4489 + (3 - dy) * 67 + (3 - dx),
                            ap=[[9 * 4489, 2], [67, 64], [1, 64]],
                        ),
                        in_=bass.AP(
                            tensor=inp[:].tensor,
                            offset=2 * pp * 4096,
                            ap=[[4096, 2], [64, 64], [1, 64]],
                        ),
                    )


            # ---------------- 1ch conv weights -> [128,128] lhsT -----------
            # rows 16*s+t (t<9) hold w.T for sample half s; zero elsewhere;
            # replicated at partition bases 32/64/96 so each pair's K=32
            # matmul has lhsT at its own base.
            convT = {}
            for name, wdram in (("c0", wc0), ("c2", wc2)):
                s9 = const.tile([64, 9], F32, tag=f"w9_{name}")
                nc.sync.dma_start(
                    out=s9[:], in_=wdram[:].rearrange("a b c d -> a (b c d)")
                )
                ct = const.tile([128, 128], F32R, tag=f"cT_{name}")
                nc.vector.memset(ct[0:32, :].bitcast(F32), 0.0)
                p9 = psmm.tile([9, 64], F32, tag="mm")
                nc.tensor.transpose(p9[:], s9[:], ident[0:64, 0:64])
                nc.scalar.activation(ct[0:9, 0:64], p9[:], AF.Copy, bias=0.0, scale=1.0)
                nc.gpsimd.dma_start(out=ct[9:18, 64:128], in_=ct[0:9, 0:64])
                for b, eng in ((32, nc.gpsimd), (64, nc.sync), (96, nc.scalar)):
                    eng.dma_start(out=ct[b : b + 32, :], in_=ct[0:32, :])
                convT[name] = ct

            # ---------------- 64ch conv weights -> block-diag lhsT ---------
            # wtap[name][:, t, :] is the [128,128] lhsT for tap t:
            #   rows 0:64  = w.T[ci,co] in cols 0:64   (sample A)
            #   rows 64:128= w.T[ci,co] in cols 64:128 (sample B)
            wtap = {}

            def build_wtap(name, wdram, scale):
                wsrc = dbl.tile([64, 576], F32, tag="c4in")  # aliased slot
                nc.sync.dma_start(
                    out=wsrc[:], in_=wdram[:].rearrange("a b c d -> a (b c d)")
                )
                wt = const.tile([128, 9, 128], F32R, tag=f"wtap_{name}")
                nc.vector.memset(wt[:].bitcast(F32), 0.0)
                for t in range(9):
                    pw = psmm.tile([64, 64], F32, tag="mm")
                    nc.tensor.transpose(pw[:], wsrc[:, t::9], ident[0:64, 0:64])
                    nc.scalar.activation(
                        wt[0:64, t, 0:64], pw[:], AF.Copy, bias=0.0, scale=scale
                    )
                nc.gpsimd.dma_start(out=wt[64:128, :, 64:128], in_=wt[0:64, :, 0:64])
                wtap[name] = wt

            build_wtap("c1", wc1, 0.25)  # 0.25: preceding avg-pool folded in

            # enc conv (64ci -> 1co): lhsT[:, t, :] is [128, 2]
            wencs = const.tile([64, 9], F32, tag="wencs")
            nc.sync.dma_start(
                out=wencs[:], in_=wen[:].rearrange("a b c d -> (a b) (c d)")
            )
            encT = const.tile([128, 9, 2], F32R, tag="encT")
            nc.vector.memset(encT[:].bitcast(F32), 0.0)
            for t in range(9):
                nc.scalar.activation(
                    encT[0:64, t, 0:1],
                    wencs[:, t : t + 1],
                    AF.Copy,
                    bias=0.0,
                    scale=0.25,  # preceding avg-pool folded in
                )
            nc.gpsimd.dma_start(out=encT[64:128, :, 1:2], in_=encT[0:64, :, 0:1])

            # ---------------- conv biases -> [128,1] (both sample halves) ---
            def bias128(dram_b, tag):
                bt = const.tile([128, 1], F32, tag=tag)
                nc.sync.dma_start(out=bt[0:64, :], in_=dram_b[:].unsqueeze(1))
                nc.sync.dma_start(out=bt[64:128, :], in_=dram_b[:].unsqueeze(1))
                return bt

            bt0 = bias128(bc0, "bt0")
            bt1 = bias128(bc1, "bt1")
            bt2 = bias128(bc2, "bt2")
            bt3 = bias128(bc3, "bt3")
            bt4 = bias128(bc4, "bt4")
            bte = const.tile([2, 1], F32, tag="bte")
            nc.sync.dma_start(
                out=bte[:],
                in_=bass.AP(tensor=ben[:].tensor, offset=0, ap=[[0, 2], [1, 1]]),
            )

            xstage = const.tile([8, 16, 16], F32, tag="xstage")

            # ================ encoder: stage-major over 4 sample pairs ======
            # conv0 phase: dense matmuls for all pairs; relu-evict (ACT) and
            # 2x2 pool (two DVE adds) chase per tile, writing straight into
            # the padded conv1 input.
            c1in_l = []
            for p in range(NPAIR):
                c1in = tri3.tile([128, 34, 34], F32R, tag="c1in")
                nc.gpsimd.memset(c1in[:, 0:1, :].bitcast(F32), 0.0)
                nc.gpsimd.memset(c1in[:, 33:34, :].bitcast(F32), 0.0)
                nc.gpsimd.memset(c1in[:, 1:33, 0:1].bitcast(F32), 0.0)
                nc.gpsimd.memset(c1in[:, 1:33, 33:34].bitcast(F32), 0.0)
                for n in range(8):
                    ps = psmm.tile([128, 4, 2, 32, 2], F32, tag="mm")
                    nc.tensor.matmul(
                        ps[:],
                        convT["c0"][32 * p : 32 * p + 18, :],
                        pat0[32 * p : 32 * p + 18, 2 + n * 8 : 10 + n * 8, 2:66],
                        start=True,
                        stop=True,
                        tile_position=(32 * p, 0),
                    )
                    ct0 = trip.tile([128, 4, 2, 32, 2], F32, tag="ct0")
                    nc.scalar.activation(ct0[:], ps[:], AF.Relu, bias=bt0)
                    tcol = tri3.tile([128, 4, 2, 32], F32, tag="tcol")
                    nc.vector.tensor_add(
                        tcol[:], ct0[:, :, :, :, 0], ct0[:, :, :, :, 1]
                    )
                    nc.vector.tensor_add(
                        c1in[:, 1 + 4 * n : 5 + 4 * n, 1:33],
                        tcol[:, :, 0, :],
                        tcol[:, :, 1, :],
                    )
                c1in_l.append(c1in)
            # conv1 phase
            ein_l = []
            for p in range(NPAIR):
                c1in = c1in_l[p]
                e_in = quad.tile([128, 18, 18], F32R, tag="e_in")
                nc.gpsimd.memset(e_in[:, 0:1, :].bitcast(F32), 0.0)
                nc.gpsimd.memset(e_in[:, 17:18, :].bitcast(F32), 0.0)
                nc.gpsimd.memset(e_in[:, 1:17, 0:1].bitcast(F32), 0.0)
                nc.gpsimd.memset(e_in[:, 1:17, 17:18].bitcast(F32), 0.0)
                for n in range(2):
                    ps = psmm.tile([128, 8, 2, 16, 2], F32, tag="mm")
                    for t, (dy, dx) in enumerate(TAPS):
                        nc.tensor.matmul(
                            ps[:],
                            wtap["c1"][:, t, :],
                            c1in[:, n * 16 + dy : n * 16 + dy + 16, dx : dx + 32]
                            ,
                            start=(t == 0),
                            stop=(t == 8),
                        )
                    ct1 = trip.tile([128, 8, 2, 16, 2], F32, tag="ct1")
                    nc.scalar.activation(ct1[:], ps[:], AF.Relu, bias=bt1)
                    tc1 = tri3.tile([128, 8, 2, 16], F32, tag="tc1")
                    nc.vector.tensor_add(
                        tc1[:], ct1[:, :, :, :, 0], ct1[:, :, :, :, 1]
                    )
                    nc.vector.tensor_add(
                        e_in[:, 1 + 8 * n : 9 + 8 * n, 1:17],
                        tc1[:, :, 0, :],
                        tc1[:, :, 1, :],
                    )
                ein_l.append(e_in)
            # enc phase
            for p in range(NPAIR):
                e_in = ein_l[p]
                pe = psmm.tile([2, 16, 16], F32, tag="mm")
                for t, (dy, dx) in enumerate(TAPS):
                    nc.tensor.matmul(
                        pe[:],
                        encT[:, t, :],
                        e_in[:, dy : dy + 16, dx : dx + 16],
                        start=(t == 0),
                        stop=(t == 8),
                    )
                estage = dbl.tile([2, 16, 16], F32, tag="estage")
                nc.scalar.activation(estage[:], pe[:], AF.Relu, bias=bte)
                nc.scalar.dma_start(out=xstage[2 * p : 2 * p + 2, :, :], in_=estage[:])

            # pre-warm the ACT tables the NTM step needs so the loads happen
            # during the encoder tail instead of inside the NTM bubble
            warm = const.tile([1, 4], F32, tag="warm")
            for wi, af in enumerate((AF.Sigmoid, AF.Tanh, AF.Exp, AF.Ln)):
                nc.scalar.activation(
                    warm[:, wi : wi + 1], ident[0:1, 0:1], af, bias=0.0
                )

            # deferred weight prep: decoder taps + NTM weights (fills the
            # PE bubble while the NTM chain runs)
            build_wtap("c3", wc3, 1.0)
            build_wtap("c4", wc4, 1.0)

            # ---------------- phase-conv weights for c3/c4 ------------------
            # conv3/conv4 inputs are 2x2 upsamples, so conv = 4 phase convs
            # with 2x2 kernels whose taps are sums of adjacent 3x3 taps:
            #   y-variants: V1 = dy1+dy2, V2 = dy0+dy1 (V0=dy0, V3=dy2 are
            #   original taps); same for x.  Phase (r=0) uses (V0,off 0),
            #   (V1,off 1); phase (r=1) uses (V2,off 1), (V3,off 2).
            PH = {0: [(0, 0), (1, 1)], 1: [(2, 1), (3, 2)]}
            phw = {}
            for name in ("c3", "c4"):
                wt3 = wtap[name][:].rearrange("p (a b) m -> p a b m", a=3)
                tA = const.tile([128, 2, 3, 128], F32R, tag=f"tA_{name}")
                tB = const.tile([128, 3, 2, 128], F32R, tag=f"tB_{name}")
                tC = const.tile([128, 2, 2, 128], F32R, tag=f"tC_{name}")
                nc.vector.tensor_add(tA[:, 0], wt3[:, 1], wt3[:, 2])
                nc.vector.tensor_add(tA[:, 1], wt3[:, 0], wt3[:, 1])
                nc.gpsimd.tensor_add(tB[:, :, 0], wt3[:, :, 1], wt3[:, :, 2])
                nc.gpsimd.tensor_add(tB[:, :, 1], wt3[:, :, 0], wt3[:, :, 1])
                nc.vector.tensor_add(tC[:, :, 0], tA[:, :, 1], tA[:, :, 2])
                nc.vector.tensor_add(tC[:, :, 1], tA[:, :, 0], tA[:, :, 1])
                phw[name] = (wt3, tA, tB, tC)

            def phsel(name, v, u):
                wt3, tA, tB, tC = phw[name]
                vy = {0: 0, 3: 2}.get(v)
                ux = {0: 0, 3: 2}.get(u)
                if vy is not None and ux is not None:
                    return wt3[:, vy, ux, :]
                if vy is not None:
                    return tB[:, vy, u - 1, :]
                if ux is not None:
                    return tA[:, v - 1, ux, :]
                return tC[:, v - 1, u - 1, :]
            # ---------------- NTM weights ----------------------------------
            # w_lstm_x rows 0:256 for gates (i, g, o); k-tiled in partitions.
            # batched: one DMA for gate i (cols 0:256), one for g+o (512:1024)
            wx = const.tile([128, 2, 768], F32R, tag="wx")
            nc.scalar.dma_start(
                out=bass.AP(
                    tensor=wx[:].tensor, offset=wx[:].offset,
                    ap=[[1536, 128], [768, 2], [1, 256]],
                ),
                in_=bass.AP(
                    tensor=wlx[:].tensor, offset=0,
                    ap=[[1024, 128], [131072, 2], [1, 256]],
                ),
            )
            nc.scalar.dma_start(
                out=bass.AP(
                    tensor=wx[:].tensor, offset=wx[:].offset + 256,
                    ap=[[1536, 128], [768, 2], [1, 512]],
                ),
                in_=bass.AP(
                    tensor=wlx[:].tensor, offset=512,
                    ap=[[1024, 128], [131072, 2], [1, 512]],
                ),
            )
            bigo = const.tile([128, 6], F32, tag="bigo")
            for j, c0 in enumerate([0, 512, 768]):
                for h2 in range(2):
                    nc.scalar.dma_start(
                        out=bigo[:, j * 2 + h2 : j * 2 + h2 + 1],
                        in_=bls[c0 + h2 * 128 : c0 + (h2 + 1) * 128].unsqueeze(1),
                    )
            # w_param gamma columns {262h+261}
            wp3 = const.tile([128, 2, 3], F32R, tag="wp3")
            for kt in range(2):
                nc.gpsimd.dma_start(
                    out=wp3[:, kt, :],
                    in_=bass.AP(
                        tensor=wpa[:].tensor,
                        offset=kt * 128 * 3108 + 261,
                        ap=[[3108, 128], [262, 3]],
                    ),
                )
            bp3 = const.tile([3, 1], F32, tag="bp3")
            nc.sync.dma_start(
                out=bp3[:],
                in_=bass.AP(tensor=bpa[:].tensor, offset=261, ap=[[262, 3], [1, 1]]),
            )
            # w_out rows 0:256 (h part) and 256:1024 (reads part, for colsums)
            wo = const.tile([128, 2, 256], F32R, tag="wo")
            nc.scalar.dma_start(
                out=bass.AP(
                    tensor=wo[:].tensor, offset=wo[:].offset,
                    ap=[[512, 128], [256, 2], [1, 256]],
                ),
                in_=bass.AP(
                    tensor=wou[:].tensor, offset=0,
                    ap=[[256, 128], [32768, 2], [1, 256]],
                ),
            )
            w2c = dbl.tile([128, 6, 256], F32R, tag="w2c")
            nc.sync.dma_start(
                out=bass.AP(
                    tensor=w2c[:].tensor, offset=w2c[:].offset,
                    ap=[[1536, 128], [256, 6], [1, 256]],
                ),
                in_=bass.AP(
                    tensor=wou[:].tensor, offset=65536,
                    ap=[[256, 128], [32768, 6], [1, 256]],
                ),
            )
            ones3 = const.tile([128, 6, 3], F32R, tag="ones3")
            nc.vector.memset(ones3[:].bitcast(F32), 0.0)
            for c in range(6):
                nc.vector.memset(ones3[:, c, c // 2 : c // 2 + 1].bitcast(F32), 1.0)
            # rhs2: rows 0:3 = per-head colsums of w_out reads part, row 3 = b_out
            rhs2 = const.tile([4, 256], F32R, tag="rhs2")
            nc.scalar.dma_start(out=rhs2[3:4, :], in_=bou[:].unsqueeze(0))
            pcs = psmm.tile([3, 256], F32, tag="mm")
            for c in range(6):
                nc.tensor.matmul(
                    pcs[:],
                    ones3[:, c, :],
                    w2c[:, c, :],
                    start=(c == 0),
                    stop=(c == 5),
                )
            nc.scalar.activation(rhs2[0:3, :], pcs[:], AF.Copy, bias=0.0, scale=1.0)
            # lhsT2: rows 0:3 = 1e-6 * S(gamma) (filled later), row 3 = 1 (bias)
            lhsT2 = const.tile([4, 8], F32R, tag="lhsT2")
            nc.vector.memset(lhsT2[:].bitcast(F32), 1.0)  # rows 0:3 rewritten before use

            # ================ NTM step (all 8 samples at once) ==============
            if debug:
                nc.sync.dma_start(out=dbg["dbg_x"][:], in_=xstage[:])
            # x^T k-tiles via PE transpose
            xT = work.tile([128, 2, 8], F32R, tag="xT")
            for kt in range(2):
                pxt = psmm.tile([128, 8], F32, tag="mm")
                nc.tensor.transpose(
                    pxt[:],
                    xstage[:].rearrange("p a b -> p (a b)")[:, kt * 128 : kt * 128 + 128],
                    ident[0:8, 0:8],
                )
                nc.scalar.activation(xT[:, kt, :], pxt[:], AF.Copy, bias=0.0, scale=1.0)
            # z = x @ Wx + b for gates i, g, o; h = sig(o) * tanh(sig(i)*tanh(g))
            zps = psmm.tile([128, 6, 8], F32, tag="mm")
            for j in range(3):
                for h2 in range(2):
                    for kt in range(2):
                        nc.tensor.matmul(
                            zps[:, 2 * j + h2, :],
                            wx[:, kt, j * 256 + h2 * 128 : j * 256 + h2 * 128 + 128],
                            xT[:, kt, :],
                            start=(kt == 0),
                            stop=(kt == 1),
                        )
            zb = work.tile([128, 6, 8], F32, tag="zb")
            bigo_b = bass.AP(
                tensor=bigo[:].tensor, offset=bigo[:].offset,
                ap=[list(d) for d in bigo[:].ap] + [[0, 8]],
            )
            nc.vector.tensor_tensor(zb[:], zps[:], bigo_b, op=ALU.add)
            si = work.tile([128, 2, 8], F32, tag="gate0")
            nc.scalar.activation(si[:], zb[:, 0:2, :], AF.Sigmoid, bias=0.0)
            tg = work.tile([128, 2, 8], F32, tag="gate1")
            nc.scalar.activation(tg[:], zb[:, 2:4, :], AF.Tanh, bias=0.0)
            so = work.tile([128, 2, 8], F32, tag="gate2")
            nc.scalar.activation(so[:], zb[:, 4:6, :], AF.Sigmoid, bias=0.0)
            ctile = work.tile([128, 2, 8], F32, tag="ctile")
            nc.vector.tensor_mul(ctile[:], si[:], tg[:])
            tct = work.tile([128, 2, 8], F32, tag="tct")
            nc.scalar.activation(tct[:], ctile[:], AF.Tanh, bias=0.0)
            h = work.tile([128, 2, 8], F32R, tag="h")
            nc.vector.tensor_mul(h[:], so[:], tct[:])
            if debug:
                nc.sync.dma_start(out=dbg["dbg_h"][:], in_=h[:])
            # gamma path: p3 = clip(h @ wp3 + bp3); q = 64*(1/64+1e-16)^gamma
            pp3 = psmm.tile([3, 8], F32, tag="mm")
            for kt in range(2):
                nc.tensor.matmul(
                    pp3[:], wp3[:, kt, :], h[:, kt, :], start=(kt == 0), stop=(kt == 1)
                )
            t1 = work.tile([3, 8], F32, tag="t1")
            nc.scalar.activation(t1[:], pp3[:], AF.Identity, bias=bp3)
            t2 = work.tile([3, 8], F32, tag="t2")
            nc.vector.tensor_scalar(t2[:], t1[:], -CLIP, CLIP, ALU.max, ALU.min)
            # softplus(p) = ln(1+exp(p)); gamma = softplus + 1,
            # q = 64*(1/64)^gamma = exp(-softplus(p)*ln64)
            eu = work.tile([3, 8], F32, tag="eu")
            nc.scalar.activation(eu[:], t2[:], AF.Exp, bias=0.0)
            ev = work.tile([3, 8], F32, tag="ev")
            nc.vector.tensor_scalar_add(ev[:], eu[:], 1.0)
            sp = work.tile([3, 8], F32, tag="sp")
            nc.scalar.activation(sp[:], ev[:], AF.Ln, bias=0.0)
            q = work.tile([3, 8], F32, tag="q")
            nc.scalar.activation(q[:], sp[:], AF.Exp, bias=0.0, scale=-LN64)
            qe = work.tile([3, 8], F32, tag="qe")
            nc.vector.tensor_scalar_add(qe[:], q[:], 1e-8)
            rec = work.tile([3, 8], F32, tag="rec")
            nc.vector.reciprocal(rec[:], qe[:])
            # lhsT2 rows 0:3 = 1e-6 * q / (q + 1e-8)
            nc.vector.scalar_tensor_tensor(
                out=lhsT2[0:3, :], in0=q[:], scalar=1e-6, in1=rec[:],
                op0=ALU.mult, op1=ALU.mult,
            )
            if debug:
                nc.sync.dma_start(out=dbg["dbg_lhst2"][:], in_=lhsT2[:])
            # out = clip(h @ w_out[:256] + reads @ w_out[256:] + b_out)
            pout = psmm.tile([8, 16, 16], F32, tag="mm")
            for kt in range(2):
                nc.tensor.matmul(
                    pout[:].rearrange("p a b -> p (a b)"),
                    h[:, kt, :],
                    wo[:, kt, :],
                    start=(kt == 0),
                    stop=False,
                )
            nc.tensor.matmul(
                pout[:].rearrange("p a b -> p (a b)"),
                lhsT2[:],
                rhs2[:],
                start=False,
                stop=True,
            )
            nc.vector.tensor_scalar(
                stg2[:, 1:17, 1:17], pout[:], -CLIP, CLIP, ALU.max, ALU.min
            )
            if debug:
                nc.sync.dma_start(out=dbg["dbg_clip"][:], in_=stg2[:, 1:17, 1:17])

            # ================ decoder: 4 sample pairs =======================
            # conv2 patches for all samples in one tile (partition 16*m+tap):
            # one stride-16 partition DMA per tap (9 DMAs total); gap
            # partitions were zeroed at setup.
            pc2 = const.tile([128, 18, 19], F32R, tag="pc2")
            for dy in range(3):
                for k in range(3):
                    for s in range(2):
                        eng = (nc.sync, nc.gpsimd, nc.scalar)[(3 * dy + k) % 3]
                        eng.dma_start(
                            out=bass.AP(
                                tensor=pc2[:].tensor,
                                offset=pc2[:].offset + (9 * s + 3 * dy + k) * 342,
                                ap=[[32 * 342, 4], [1, 341]],
                            ),
                            in_=bass.AP(
                                tensor=stg2[:].tensor,
                                offset=stg2[:].offset + s * 399 + dy * 19 + k,
                                ap=[[2 * 399, 4], [1, 341]],
                            ),
                        )
            for p in range(NPAIR):
                # --- conv2: K=18 single matmul, N=256 -> v2 [128,18,18] pad
                ps2 = psmm.tile([128, 16, 16], F32, tag="mm")
                nc.tensor.matmul(
                    ps2[:],
                    convT["c2"][32 * p : 32 * p + 18, :],
                    pc2[32 * p : 32 * p + 18, 0:16, 0:16],
                    start=True,
                    stop=True,
                    tile_position=(32 * p, 0),
                )
                v2 = quad.tile([128, 18, 18], F32R, tag="v2")
                nc.gpsimd.memset(v2[:, 0:1, :].bitcast(F32), 0.0)
                nc.gpsimd.memset(v2[:, 17:18, :].bitcast(F32), 0.0)
                nc.gpsimd.memset(v2[:, 1:17, 0:1].bitcast(F32), 0.0)
                nc.gpsimd.memset(v2[:, 1:17, 17:18].bitcast(F32), 0.0)
                nc.scalar.activation(v2[:, 1:17, 1:17], ps2[:], AF.Relu, bias=bt2)
                # --- conv3 as 4 phase convs (input is 2x2 upsample of v2)
                v3 = dbl.tile([128, 17, 2, 17, 2], F32R, tag="v3")
                nc.vector.memset(v3[:, 0, 0, :, :].bitcast(F32), 0.0)       # row 0
                nc.vector.memset(v3[:, 16, 1, :, :].bitcast(F32), 0.0)      # row 33
                nc.vector.memset(v3[:, :, :, 0, 0].bitcast(F32), 0.0)       # col 0
                nc.vector.memset(v3[:, :, :, 16, 1].bitcast(F32), 0.0)      # col 33
                for ph, (ry, rx) in enumerate([(0, 0), (0, 1), (1, 0), (1, 1)]):
                    ps = psmm.tile([128, 16, 16], F32, tag="mm")
                    i = 0
                    for v, yo in PH[ry]:
                        for u, xo in PH[rx]:
                            nc.tensor.matmul(
                                ps[:],
                                phsel("c3", v, u),
                                v2[:, yo : yo + 16, xo : xo + 16],
                                start=(i == 0),
                                stop=(i == 3),
                            )
                            i += 1
                    dst = v3[:, ry : 16 + ry, 1 - ry, rx : 16 + rx, 1 - rx]
                    if ph % 2 == 0:
                        nc.scalar.activation(dst, ps[:], AF.Relu, bias=bt3)
                    else:
                        nc.vector.tensor_scalar(
                            dst, ps[:], bt3[:], 0.0, ALU.add, ALU.max
                        )
                v3f = v3[:].rearrange("p r a c b -> p (r a) (c b)")
                if debug and p == 0:
                    nc.sync.dma_start(out=dbg["dbg_c3in"][:], in_=v3f)
                # --- conv4 as 4 phase convs (input is 2x2 upsample of v3)
                c4out = dbl.tile([128, 32, 2, 32, 2], F32, tag="c4out")
                ei = 0
                for sy in (0, 1):
                    for h in (0, 1):
                        for sx in (0, 1):
                            ps = psmm.tile([128, 16, 32], F32, tag="mm")
                            i = 0
                            for v, yo in PH[sy]:
                                for u, xo in PH[sx]:
                                    nc.tensor.matmul(
                                        ps[:],
                                        phsel("c4", v, u),
                                        v3f[:, yo + 16 * h : yo + 16 * h + 16, xo : xo + 32],
                                        start=(i == 0),
                                        stop=(i == 3),
                                    )
                                    i += 1
                            dst = c4out[:, 16 * h : 16 * h + 16, sy, :, sx]
                            if ei % 2 == 0:
                                nc.scalar.activation(dst, ps[:], AF.Relu, bias=bt4)
                            else:
                                nc.vector.tensor_scalar(
                                    dst, ps[:], bt4[:], 0.0, ALU.add, ALU.max
                                )
                            ei += 1
                        if p == NPAIR - 1:
                            # fine-grained row stores so the tail drains early:
                            # rows 32h+sy(+2k) for k<16 are complete now
                            for s01 in range(2):
                                for c in range(4):
                                    eng = (nc.sync, nc.gpsimd, nc.scalar)[(s01 * 4 + c) % 3]
                                    eng.dma_start(
                                        out=bass.AP(
                                            tensor=out[:].tensor,
                                            offset=(2 * p + s01) * 262144
                                            + (16 * c) * 4096
                                            + (32 * h + sy) * 64,
                                            ap=[[4096, 16], [128, 16], [1, 64]],
                                        ),
                                        in_=c4out[
                                            64 * s01 + 16 * c : 64 * s01 + 16 * c + 16,
                                            16 * h : 16 * h + 16, sy, :, :,
                                        ],
                                    )
                if p < NPAIR - 1:
                    c4v = c4out[:].rearrange("p r a c b -> p (r a) (c b)")
                    for s01 in range(2):
                        for c in range(4):
                            eng = (nc.sync, nc.gpsimd)[(s01 * 4 + c) % 2]
                            eng.dma_start(
                                out=out[2 * p + s01, 16 * c : 16 * c + 16, :, :],
                                in_=c4v[64 * s01 + 16 * c : 64 * s01 + 16 * c + 16, :, :],
                            )


    nc.compile()
    return nc


_NC_CACHE = {}
LAST_RESULT = None

WEIGHT_NAMES = [
    "w_conv0", "b_conv0", "w_conv1", "b_conv1", "w_enc", "b_enc",
    "w_conv2", "b_conv2", "w_conv3", "b_conv3", "w_conv4", "b_conv4",
    "w_lstm_x", "b_lstm", "w_param", "b_param", "w_out", "b_out",
]


def kernel(**inputs):
    global LAST_RESULT
    from concourse.bass_utils import run_bass_kernel_spmd

    debug = bool(int(os.environ.get("KDEBUG", "0")))
    key = ("nc", debug)
    if key not in _NC_CACHE:
        _NC_CACHE[key] = build_nc(debug=debug)
    nc = _NC_CACHE[key]

    xs = np.ascontiguousarray(np.asarray(inputs["inputs"], dtype=np.float32))
    weights = {
        k: np.ascontiguousarray(np.asarray(inputs[k], dtype=np.float32))
        for k in WEIGHT_NAMES
    }
    in_maps = []
    for c in range(N_CORES):
        m = dict(weights)
        m["inputs"] = xs[c * B_CORE : (c + 1) * B_CORE]
        in_maps.append(m)

    res = run_bass_kernel_spmd(nc, in_maps, core_ids=list(range(N_CORES)))
    LAST_RESULT = res
    return np.concatenate([r["out"] for r in res.results], axis=0)


if __name__ == "__main__":
    nc = build_nc()
    print("built ok")

